# revision 59
# baseline (speedup 1.0000x reference)
"""Trainium2 Bass kernel for nn_DecoderBlock (self-attn + cross-attn + FFN).

Sharding: sequence-parallel, no collectives. 8 cores = 2 batches x 4
L-chunks of 512 tokens. Windowed self-attention (W=64) needs only a
64-row halo; cross-attention K/V are recomputed per core from the full
`mem` of that core's batch.

On-chip layout: activations are feature-major [d_partition, token_free]
so every projection is matmul(out[do,t], lhsT=W[k,do], rhs=x[k,t]) with
weights in natural [d_in, d_out] layout. Attention scores are s-major
[key_part, query_free]; softmax row-sums come from an appended ones
column in token-major V. Softmax normalization is fully on-chip:
row-sum -> 1/x via exp(-ln(x)) on ACT -> gpsimd partition_broadcast ->
one DVE multiply straight into the pair-stacked av2 tile. LayerNorm
stats use ones-vector matmuls (float32r); rstd = exp(-0.5*ln(var+eps));
mean/rstd broadcast across partitions on gpsimd. ALiBi bias +
causal-window mask enter via per-pair additive score templates.

All hot matmuls are padded to full (128,128) PE tiles: partial-tile
matmuls (tile_size != (128,128)) run on a slow clock path, so score
matmuls contract over zero-padded 128-dim q vectors and AV matmuls use
128-column [v | ones | 0] stationary blocks.
"""

import sys

sys.path.insert(0, "/opt/trn_rl_repo")

import numpy as np
import ml_dtypes

import concourse.bass as bass
import concourse.mybir as mybir
import concourse.tile as tile
from concourse import bacc
from concourse.bass_utils import run_bass_kernel_spmd

BF16 = mybir.dt.bfloat16
F32 = mybir.dt.float32
F32R = mybir.dt.float32r
AF = mybir.ActivationFunctionType
ALU = mybir.AluOpType

B, L, MEM, D, H, FF, W = 2, 2048, 2048, 1024, 16, 4096, 64
DH = D // H  # 64
P = 128
KS = D // P  # 8
NFF = FF // P  # 32
TCH = 512
HALO = 64
SK = HALO + TCH + 64  # padded self-attn key length (640)
NEG = -30000.0
NBIAS = ["sbq", "sbk", "sbo", "cbq", "cbk", "cbo", "b2",
         "g1", "be1", "g2", "be2", "g3", "be3"]


def _mm(nc, out, lhsT, rhs, start, stop):
    return nc.tensor.matmul(out, lhsT, rhs, start=start, stop=stop)


def build(nc):
    dt = nc.dram_tensor
    io = {}
    io["xb"] = dt("xb", [P, KS, HALO + TCH], BF16, kind="ExternalInput")
    io["xf"] = dt("xf", [P, KS, TCH], F32, kind="ExternalInput")
    io["memf"] = dt("memf", [P, KS, MEM], BF16, kind="ExternalInput")
    for n in ["swq", "swk", "swv", "cwq"]:
        io[n] = dt(n, [P, KS, D], BF16, kind="ExternalInput")
    for n in ["swo", "cwo"]:
        io[n] = dt(n, [P, KS, D], BF16, kind="ExternalInput")
    # per-core slices: this core's 4 cross-attn heads (256 features)
    io["cwk_my"] = dt("cwk_my", [P, KS, 2 * P], BF16, kind="ExternalInput")
    io["cwv_my"] = dt("cwv_my", [P, KS, 2 * P], BF16, kind="ExternalInput")
    io["cbk_my"] = dt("cbk_my", [P, 2], F32, kind="ExternalInput")
    io["w1"] = dt("w1", [P, KS, FF], BF16, kind="ExternalInput")
    io["w2"] = dt("w2", [P, NFF, D], BF16, kind="ExternalInput")
    # all per-feature bias/scale vectors packed into one tensor: 13 KS-wide
    # blocks then b1 (NFF wide)
    io["biases"] = dt("biases", [P, 13 * KS + NFF], F32, kind="ExternalInput")
    io["tmplP0"] = dt("tmplP0", [P, KS, 4 * P], BF16, kind="ExternalInput")
    io["tmplP"] = dt("tmplP", [P, KS, 4 * P], BF16, kind="ExternalInput")
    io["out"] = dt("out", [P, KS, TCH], F32, kind="ExternalOutput")

    with tile.TileContext(nc) as tc:
        _build_tc(nc, tc, io)
    return nc


def _build_tc(nc, tc, io):
    import contextlib

    with contextlib.ExitStack() as ctx:
        consts = ctx.enter_context(tc.tile_pool(name="consts", bufs=1))
        wp = ctx.enter_context(tc.tile_pool(name="wp", bufs=3))
        # PSUM: tag "ps" [P,512] x2 (1 bank each) + tag "ps2" [P,1024] x2
        # (2 banks each) + tag "pav" [P,512] x2 = 8 banks exactly
        ps = ctx.enter_context(tc.tile_pool(name="ps", bufs=2, space="PSUM"))
        sm = ctx.enter_context(tc.tile_pool(name="sm", bufs=2))
        residp = ctx.enter_context(tc.tile_pool(name="residp", bufs=2))
        xqp = ctx.enter_context(tc.tile_pool(name="xqp", bufs=1))
        avp = ctx.enter_context(tc.tile_pool(name="avp", bufs=1))
        qp = ctx.enter_context(tc.tile_pool(name="qp", bufs=1))

        # padded-q tiles: even head dims on partitions 0:64 (rest zero),
        # odd head dims on partitions 64:128 (rest zero). Shared between
        # the self- and cross-attention phases.
        qe = qp.tile([P, KS, TCH], BF16, tag="qe")
        qo = qp.tile([P, KS, TCH], BF16, tag="qo")
        nc.gpsimd.memset(qe[DH:P, :, :], 0.0)
        nc.gpsimd.memset(qo[0:DH, :, :], 0.0)

        biases = consts.tile([P, 13 * KS + NFF], F32)
        nc.sync.dma_start(biases[:], io["biases"][:])
        sb = {n: biases[:, i * KS : (i + 1) * KS] for i, n in enumerate(NBIAS)}
        sb["b1"] = biases[:, 13 * KS :]
        cbkm = consts.tile([P, 2], F32)
        nc.sync.dma_start(cbkm[:], io["cbk_my"][:])
        sb["cbk_my"] = cbkm[:]

        # ones128: column 0 is ones, rest zeros -> full (128,128) LN matmul
        ones128_f32 = consts.tile([P, P], F32)
        nc.vector.memset(ones128_f32[:], 0.0)
        nc.vector.memset(ones128_f32[:, 0:1], 1.0)
        ones128 = consts.tile([P, P], F32R)
        nc.vector.tensor_copy(out=ones128[:], in_=ones128_f32[:])
        eps_t = consts.tile([P, 1], F32)
        nc.vector.memset(eps_t[:], 1e-5)

        NSC = MEM // P  # 16
        cp_stack_pools = {}
        KLEN = 2 * MEM  # 2 pair-blocks of this core's 4 heads
        VLEN = NSC * 4 * (DH + 1)  # 4160
        cc_in_k = nc.dram_tensor("cc_in_k", [P, KLEN], BF16)
        cc_out_k = nc.dram_tensor("cc_out_k", [4, P, KLEN], BF16)
        cc_in_v = nc.dram_tensor("cc_in_v", [P, VLEN], BF16)
        cc_out_v = nc.dram_tensor("cc_out_v", [4, P, VLEN], BF16)
        GROUPS = [[0, 1, 2, 3], [4, 5, 6, 7]]

        # cross-attn K: each core computes its own 4 heads (256 features)
        # over the full mem, then the 4 cores of a batch all-gather (small
        # and early -> fully hidden). V is recomputed locally per group as
        # PE filler for the ACT-bound cross-attention loop.
        def k_local(kvp):
            wkt = kvp.tile([P, KS, 2 * P], BF16, tag="wkt")
            nc.sync.dma_start(wkt[:], io["cwk_my"][:])
            mcs = []
            for qtr in range(4):
                mc = kvp.tile([P, KS, 512], BF16, tag=f"memc{qtr}")
                nc.sync.dma_start(mc[:], io["memf"][:, :, bass.ts(qtr, 512)])
                mcs.append(mc)
            k_part = kvp.tile([P, 2, MEM], BF16, tag="k_part")
            for scn in range(4):
                mc, c0 = mcs[scn], 0
                for do in range(2):
                    pt = ps.tile([P, 512], F32, tag="ps")
                    for k in range(KS):
                        last_mm = _mm(nc, pt[:], wkt[:, k, bass.ts(do, P)],
                                      mc[:, k, c0 : c0 + 512], k == 0, k == KS - 1)
                    nc.scalar.activation(
                        k_part[:, do, bass.ts(scn, 512)], pt[:], AF.Identity,
                        bias=sb["cbk_my"][:, do : do + 1], scale=1.0)
            nc.gpsimd.dma_start(cc_in_k[:], k_part[:])
            nc.gpsimd.collective_compute(
                "AllGather", ALU.bypass, replica_groups=GROUPS,
                ins=[cc_in_k.ap()], outs=[cc_out_k.ap()])
            wvt = kvp.tile([P, KS, 2 * P], BF16, tag="wvt")
            nc.sync.dma_start(wvt[:], io["cwv_my"][:])
            v_part = kvp.tile([P, NSC, 4 * (DH + 1)], BF16, tag="v_part")
            vv = v_part[:].rearrange("p s (h c) -> p s h c", c=DH + 1)
            nc.gpsimd.memset(vv[:, :, :, DH : DH + 1], 1.0)
            for scn in range(4):
                mc, c0 = mcs[scn], 0
                for si in range(4):
                    sc = scn * 4 + si
                    pt = ps.tile([P, 512], F32, tag="ps")
                    for k in range(KS):
                        _mm(nc, pt[:, : 2 * P],
                            mc[:, k, c0 + si * P : c0 + (si + 1) * P],
                            wvt[:, k, :], k == 0, k == KS - 1)
                    dst = v_part[:, sc, :].rearrange(
                        "p (h c) -> p h c", c=DH + 1)[:, :, 0:DH]
                    nc.vector.tensor_copy(
                        out=dst,
                        in_=pt[:, : 2 * P].rearrange("p (h c) -> p h c", c=DH))
            nc.gpsimd.dma_start(cc_in_v[:], v_part[:])
            nc.gpsimd.collective_compute(
                "AllGather", ALU.bypass, replica_groups=GROUPS,
                ins=[cc_in_v.ap()], outs=[cc_out_v.ap()])

        def fill_v8(g, cp2, after=None):
            # per-head 65-stride blocks [v(64) | ones(1)] + a 63-col zero
            # tail; the AV lhsT window [h*65 : h*65+128] overlaps the next
            # head, whose contribution lands in psum rows 65:127 (never read)
            VW = 8 * (DH + 1) + DH - 1  # 583
            v8 = cp2.tile([P, NSC, VW], BF16, tag="v8", bufs=2)
            d1 = nc.sync.dma_start(
                v8[:, :, 0 : 4 * (DH + 1)],
                cc_out_v[2 * g].rearrange("p (s c) -> p s c", c=4 * (DH + 1)))
            d2 = nc.sync.dma_start(
                v8[:, :, 4 * (DH + 1) : 8 * (DH + 1)],
                cc_out_v[2 * g + 1].rearrange("p (s c) -> p s c", c=4 * (DH + 1)))
            nc.gpsimd.memset(v8[:, :, 8 * (DH + 1) :], 0.0)
            if after is not None:
                tile.add_dep_helper(d1.ins, after.ins, reason="defer fill")
                tile.add_dep_helper(d2.ins, after.ins, reason="defer fill")
            return v8

        def fill_k8(g, after=None):
            k8 = cp_stack_pools["kp"].tile([P, 4, MEM], BF16, tag="k8", bufs=2)
            d1 = nc.sync.dma_start(k8[:, 0:2, :], cc_out_k[2 * g, :, :])
            d2 = nc.sync.dma_start(k8[:, 2:4, :], cc_out_k[2 * g + 1, :, :])
            if after is not None:
                tile.add_dep_helper(d1.ins, after.ins, reason="defer fill")
                tile.add_dep_helper(d2.ins, after.ins, reason="defer fill")
            return k8


        # self-attn input loaded up front (qp pool outlives kvp) so its DMA
        # overlaps the local K/V compute
        xb_sb = qp.tile([P, KS, HALO + TCH], BF16, tag="xb")
        nc.sync.dma_start(xb_sb[:], io["xb"][:])

        # local K/V compute + all-gather launch first: the gather transfer
        # hides behind the whole self-attention phase
        with tc.tile_pool(name="kvp", bufs=1) as kvp:
            k_local(kvp)

        # ================= SELF-ATTENTION =================
        with tc.tile_pool(name="selfp", bufs=1) as sp, \
             tc.tile_pool(name="exps", bufs=2) as epo:

            for do in range(KS):
                wt = wp.tile([P, KS, P], BF16, tag="wproj")
                nc.sync.dma_start(wt[:], io["swq"][:, :, bass.ts(do, P)])
                pt = ps.tile([P, 512], F32, tag="ps")
                for k in range(KS):
                    _mm(nc, pt[:], wt[:, k, :], xb_sb[:, k, HALO:],
                        k == 0, k == KS - 1)
                nc.scalar.activation(qe[0:DH, do, :], pt[0:DH, :], AF.Identity,
                                     bias=sb["sbq"][0:DH, do : do + 1], scale=1.0)
                nc.vector.tensor_scalar(out=qo[DH:P, do, :], in0=pt[DH:P, :],
                                        scalar1=sb["sbq"][DH:P, do : do + 1],
                                        scalar2=None, op0=ALU.add)

            tp0 = sp.tile([P, KS, 4 * P], BF16)
            tp1 = sp.tile([P, KS, 4 * P], BF16)
            nc.sync.dma_start(tp0[:], io["tmplP0"][:])
            nc.sync.dma_start(tp1[:], io["tmplP"][:])

            # whole swk preloaded in one DMA: its tail chunks otherwise
            # stream into the K/V all-gather window and starve
            swk_sb = sp.tile([P, KS, D], BF16)
            nc.sync.dma_start(swk_sb[:], io["swk"][:])
            k_sb = sp.tile([P, KS, SK], BF16)
            nc.vector.memset(k_sb[:, :, HALO + TCH :], 0.0)
            for do in range(KS):
                pt = ps.tile([P, 512], F32, tag="ps")
                pt2 = ps.tile([P, 512], F32, tag="ps")
                for k in range(KS):
                    _mm(nc, pt[:], swk_sb[:, k, bass.ts(do, P)],
                        xb_sb[:, k, 0:512], k == 0, k == KS - 1)
                for k in range(KS):
                    _mm(nc, pt2[:, :HALO], swk_sb[:, k, bass.ts(do, P)],
                        xb_sb[:, k, 512:576], k == 0, k == KS - 1)
                nc.scalar.activation(k_sb[:, do, 0:512], pt[:], AF.Identity,
                                     bias=sb["sbk"][:, do : do + 1], scale=1.0)
                nc.scalar.activation(k_sb[:, do, 512:576], pt2[:, :HALO],
                                     AF.Identity,
                                     bias=sb["sbk"][:, do : do + 1], scale=1.0)

            # token-major V with per-head 65-stride blocks [v | ones] and a
            # 63-col zero tail (AV lhsT windows overlap the next head)
            NSS = 5
            VW = H * (DH + 1) + DH - 1  # 1103
            v_sb = sp.tile([P, NSS, VW], BF16)
            vv = v_sb[:, :, 0 : H * (DH + 1)].rearrange("p s (h c) -> p s h c",
                                                        c=DH + 1)
            nc.vector.memset(vv[:, :, :, DH : DH + 1], 1.0)
            nc.vector.memset(v_sb[:, :, H * (DH + 1) :], 0.0)
            # last key block only has 64 valid token rows; zero the rest
            nc.vector.memset(v_sb[DH:P, NSS - 1, :], 0.0)
            for dv in range(2):
                wt = wp.tile([P, KS, 512], BF16, tag="wv", bufs=1)
                nc.sync.dma_start(wt[:], io["swv"][:, :, bass.ts(dv, 512)])
                for sc in range(NSS):
                    n_s = min(P, HALO + TCH - sc * P)
                    pt = ps.tile([P, 512], F32, tag="ps")
                    for k in range(KS):
                        _mm(nc, pt[:n_s, :], xb_sb[:, k, sc * P : sc * P + n_s],
                            wt[:, k, :], k == 0, k == KS - 1)
                    dst = v_sb[:n_s, sc,
                               dv * 8 * (DH + 1) : (dv * 8 + 8) * (DH + 1)]
                    dst = dst.rearrange("p (h c) -> p h c", c=DH + 1)[:, :, 0:DH]
                    nc.vector.tensor_copy(
                        out=dst,
                        in_=pt[:n_s, :].rearrange("p (h c) -> p h c", c=DH))

            # prefetch the whole self O-projection weight before the
            # K/V all-gather saturates the DMA fabric
            swo_sb = sp.tile([P, KS, D], BF16)
            nc.sync.dma_start(swo_sb[:], io["swo"][:])
            # residual input is only consumed at the self O-proj adds; keep
            # its 2MB off the pre-gather queue so weights land first
            xf_sb = residp.tile([P, KS, TCH], F32, tag="resid")
            nc.sync.dma_start(xf_sb[:], io["xf"][:])

            av2 = avp.tile([P, KS, TCH], BF16, tag="av2")
            for j in range(H // 2):
                he, ho = 2 * j, 2 * j + 1
                pav_e = ps.tile([P, 512], F32, tag="pav", bufs=2)
                pav_o = ps.tile([P, 512], F32, tag="pav", bufs=2)
                for tcn in range(4):
                    T0 = tcn * P
                    q_eb = qe[:, j, T0 : T0 + P]
                    q_ob = qo[:, j, T0 : T0 + P]
                    kA = k_sb[:, j, T0 : T0 + P]
                    kB = k_sb[:, j, T0 + P : T0 + 2 * P]
                    pS = ps.tile([P, 512], F32, tag="ps2", bufs=2)
                    _mm(nc, pS[:, 0:P], kA, q_eb, True, True)
                    _mm(nc, pS[:, P : 2 * P], kB, q_eb, True, True)
                    _mm(nc, pS[:, 2 * P : 3 * P], kA, q_ob, True, True)
                    _mm(nc, pS[:, 3 * P :], kB, q_ob, True, True)
                    tp = tp0 if tcn == 0 else tp1
                    ef = epo.tile([P, 512], F32, tag="ef")
                    nc.vector.tensor_tensor(ef[:], pS[:], tp[:, j, :], ALU.add)
                    e = epo.tile([P, 512], BF16, tag="eb")
                    nc.scalar.activation(e[:], ef[:], AF.Exp)
                    we = he * (DH + 1)
                    wo = ho * (DH + 1)
                    _mm(nc, pav_e[:, T0 : T0 + P],
                        v_sb[:, tcn, we : we + P], e[:, 0:P], True, False)
                    _mm(nc, pav_e[:, T0 : T0 + P],
                        v_sb[:, tcn + 1, we : we + P], e[:, P : 2 * P],
                        False, True)
                    _mm(nc, pav_o[:, T0 : T0 + P],
                        v_sb[:, tcn, wo : wo + P], e[:, 2 * P : 3 * P],
                        True, False)
                    _mm(nc, pav_o[:, T0 : T0 + P],
                        v_sb[:, tcn + 1, wo : wo + P], e[:, 3 * P :],
                        False, True)
                _head_norm(nc, sm, pav_e, he, av2)
                _head_norm(nc, sm, pav_o, ho, av2)

            _oproj(nc, ps, wp, sm, av2, io, "swo", sb["sbo"], xf_sb,
                   wo_sb=swo_sb)

        cp_stack = contextlib.ExitStack()
        cp2 = cp_stack.enter_context(tc.tile_pool(name="cp2", bufs=1))
        epc = cp_stack.enter_context(tc.tile_pool(name="expc", bufs=4))
        cp_stack_pools["kp"] = cp_stack.enter_context(
            tc.tile_pool(name="kp", bufs=1))

        x1_sb = residp.tile([P, KS, TCH], F32, tag="resid")
        x1b = xqp.tile([P, KS, TCH], BF16, tag="xq")
        ln1_last = _ln(nc, ps, sm, xf_sb, x1_sb, sb["g1"], sb["be1"], ones128,
                       eps_t, out_bf=x1b, fold_affine=True)
        # fills pinned after LN1 so their cc_out waits can't occupy the sync
        # DMA queue during the self-attention phase
        k8_g0 = fill_k8(0, after=ln1_last)
        v8_g0 = fill_v8(0, cp2, after=ln1_last)

        # ================= CROSS-ATTENTION =================
        for do in range(KS):
            wt = wp.tile([P, KS, P], BF16, tag="wproj")
            nc.sync.dma_start(wt[:], io["cwq"][:, :, bass.ts(do, P)])
            pt = ps.tile([P, 512], F32, tag="ps")
            for k in range(KS):
                _mm(nc, pt[:], wt[:, k, :], x1b[:, k, :], k == 0, k == KS - 1)
            nc.scalar.activation(qe[0:DH, do, :], pt[0:DH, :], AF.Identity,
                                 bias=sb["cbq"][0:DH, do : do + 1], scale=1.0)
            nc.vector.tensor_scalar(out=qo[DH:P, do, :], in0=pt[DH:P, :],
                                    scalar1=sb["cbq"][DH:P, do : do + 1],
                                    scalar2=None, op0=ALU.add)

        av2 = avp.tile([P, KS, TCH], BF16, tag="av2")
        kv = (k8_g0, v8_g0)
        for g in range(2):
            k8, v8 = kv
            for j in range(4):
                he, ho = g * 8 + 2 * j, g * 8 + 2 * j + 1
                q_e = qe[:, g * 4 + j, :]
                q_o = qo[:, g * 4 + j, :]
                pav_e = ps.tile([P, 512], F32, tag="pav", bufs=2)
                pav_o = ps.tile([P, 512], F32, tag="pav", bufs=2)
                for sc in range(NSC):
                    pS = ps.tile([P, 2 * TCH], F32, tag="ps2", bufs=2)
                    _mm(nc, pS[:, 0:TCH], k8[:, j, bass.ts(sc, P)], q_e,
                        True, True)
                    _mm(nc, pS[:, TCH:], k8[:, j, bass.ts(sc, P)], q_o,
                        True, True)
                    e2 = epc.tile([P, 2 * TCH], BF16, tag="ec")
                    nc.scalar.activation(e2[:], pS[:], AF.Exp)
                    vA_e = v8[:, sc, (2 * j) * (DH + 1) : (2 * j) * (DH + 1) + P]
                    vA_o = v8[:, sc,
                              (2 * j + 1) * (DH + 1) : (2 * j + 1) * (DH + 1) + P]
                    _mm(nc, pav_e[:], vA_e, e2[:, 0:TCH], sc == 0, sc == NSC - 1)
                    _mm(nc, pav_o[:], vA_o, e2[:, TCH:], sc == 0, sc == NSC - 1)
                _head_norm(nc, sm, pav_e, he, av2)
                _head_norm(nc, sm, pav_o, ho, av2)
                if g == 0 and j == 3:
                    kv = (fill_k8(1), fill_v8(1, cp2))

        _oproj(nc, ps, wp, sm, av2, io, "cwo", sb["cbo"], x1_sb)
        cp_stack.close()

        x2_sb = residp.tile([P, KS, TCH], F32, tag="resid")
        x2b = xqp.tile([P, KS, TCH], BF16, tag="xq")
        _ln(nc, ps, sm, x1_sb, x2_sb, sb["g2"], sb["be2"], ones128, eps_t,
            out_bf=x2b, fold_affine=True)

        # ================= FFN =================
        with tc.tile_pool(name="ffnp", bufs=1) as fp, \
             tc.tile_pool(name="w2p", bufs=2) as w2p:
            h1 = fp.tile([P, NFF, TCH], BF16)
            for fc in range(NFF):
                wt = wp.tile([P, KS, P], BF16, tag="wproj")
                nc.sync.dma_start(wt[:], io["w1"][:, :, bass.ts(fc, P)])
                pt = ps.tile([P, 512], F32, tag="ps")
                for k in range(KS):
                    _mm(nc, pt[:], wt[:, k, :], x2b[:, k, :], k == 0, k == KS - 1)
                nc.scalar.activation(h1[:, fc, :], pt[:], AF.Gelu,
                                     bias=sb["b1"][:, fc : fc + 1], scale=1.0)
            for do in range(KS):
                wt = w2p.tile([P, NFF, P], BF16, tag="w2t")
                nc.sync.dma_start(wt[:], io["w2"][:, :, bass.ts(do, P)])
                pt = ps.tile([P, 512], F32, tag="ps")
                for k in range(NFF):
                    _mm(nc, pt[:], wt[:, k, :], h1[:, k, :], k == 0, k == NFF - 1)
                ft = sm.tile([P, TCH], F32, tag="t2k")
                nc.vector.tensor_scalar(out=ft[:], in0=pt[:],
                                        scalar1=sb["b2"][:, do : do + 1],
                                        scalar2=None, op0=ALU.add)
                nc.vector.tensor_tensor(x2_sb[:, do, :], ft[:], x2_sb[:, do, :],
                                        ALU.add)

        out_sb = residp.tile([P, KS, TCH], F32, tag="resid")
        _ln(nc, ps, sm, x2_sb, out_sb, sb["g3"], sb["be3"], ones128, eps_t)
        for k in range(KS):
            nc.sync.dma_start(io["out"][:, k, :], out_sb[:, k, :])


def _head_norm(nc, sm, pav, h, av2):
    """Softmax-normalize one head's AV block (rows 0:64 of pav, row sums in
    row 64) into its pair-stacked position in av2. Fully on-chip: approx
    1/x on DVE, partition broadcast on gpsimd. AV and the sums row are
    staged to SBUF immediately so the psum bank frees early."""
    avs = sm.tile([DH, TCH], BF16, tag="avs", bufs=4)
    nc.vector.tensor_copy(out=avs[:], in_=pav[0:DH, :])
    rs = sm.tile([1, TCH], F32, tag="rsrow", bufs=2)
    nc.vector.tensor_copy(out=rs[0:1, :], in_=pav[DH : DH + 1, :])
    rcp_row = sm.tile([1, TCH], F32, tag="rcprow", bufs=2)
    nc.vector.reciprocal_approx_fast(out=rcp_row[:], in_=rs[:])
    rcp = sm.tile([DH, TCH], F32, tag="rcph", bufs=2)
    nc.gpsimd.partition_broadcast(rcp[:], rcp_row[0:1, :])
    nc.vector.tensor_tensor(av2[(h % 2) * DH : ((h % 2) + 1) * DH, h // 2, :],
                            avs[:], rcp[:], ALU.mult)


def _oproj(nc, ps, wp, sm, av2, io, wo_name, bo_sb, res_sb, wo_sb=None):
    """Standard K=128 projection of the pair-stacked av2 with wo in natural
    layout; bias + residual add into res_sb in place."""
    for do in range(KS):
        if wo_sb is None:
            wt = wp.tile([P, KS, P], BF16, tag="wproj")
            nc.sync.dma_start(wt[:], io[wo_name][:, :, bass.ts(do, P)])
        pt = ps.tile([P, 512], F32, tag="ps")
        for k in range(KS):
            lhs = (wo_sb[:, k, bass.ts(do, P)] if wo_sb is not None
                   else wt[:, k, :])
            _mm(nc, pt[:], lhs, av2[:, k, :], k == 0, k == KS - 1)
        st = sm.tile([P, TCH], F32, tag="t2k")
        nc.vector.tensor_scalar(out=st[:], in0=pt[:],
                                scalar1=bo_sb[:, do : do + 1], scalar2=None,
                                op0=ALU.add)
        nc.vector.tensor_tensor(res_sb[:, do, :], st[:], res_sb[:, do, :], ALU.add)


def _ln(nc, ps, sm, src_sb, out_sb, g, be, ones128, eps_t, out_bf=None,
        fold_affine=False):
    """out = LN(src) over the feature (partition) axis, feature-major.
    Sum/sumsq via full-tile ones matmuls (row 0 of psum); mean/rstd
    broadcast across partitions on gpsimd."""
    psum = ps.tile([P, 512], F32, tag="ps")
    psq = ps.tile([P, 512], F32, tag="ps")
    for k in range(KS):
        xr = sm.tile([P, TCH], F32R, tag="lnxr", bufs=1)
        nc.vector.tensor_copy(out=xr[:], in_=src_sb[:, k, :])
        _mm(nc, psum[:], ones128, xr[:], k == 0, k == KS - 1)
        sq = sm.tile([P, TCH], F32R, tag="lnsq", bufs=1)
        nc.vector.tensor_tensor(sq[:], xr[:], xr[:], ALU.mult)
        _mm(nc, psq[:], ones128, sq[:], k == 0, k == KS - 1)
    mean = sm.tile([1, TCH], F32, tag="lnm", bufs=1)
    ex2 = sm.tile([1, TCH], F32, tag="lnv", bufs=1)
    nc.vector.tensor_scalar(out=mean[:], in0=psum[:1, :], scalar1=1.0 / D,
                            scalar2=None, op0=ALU.mult)
    nc.vector.tensor_scalar(out=ex2[:], in0=psq[:1, :], scalar1=1.0 / D,
                            scalar2=None, op0=ALU.mult)
    var = sm.tile([1, TCH], F32, tag="lnvar", bufs=1)
    nc.vector.tensor_tensor(var[:], mean[:], mean[:], ALU.mult)
    nc.vector.tensor_tensor(var[:], ex2[:], var[:], ALU.subtract)
    rstd = sm.tile([1, TCH], F32, tag="lnstd", bufs=1)
    # rstd = sqrt(1/(var+eps)): DVE approx reciprocal, then ACT sqrt
    rvar = sm.tile([1, TCH], F32, tag="lnrv", bufs=1)
    nc.vector.tensor_scalar(out=rvar[:], in0=var[:], scalar1=eps_t[:1, :],
                            scalar2=None, op0=ALU.add)
    nc.vector.reciprocal_approx_fast(out=rvar[:], in_=rvar[:])
    nc.scalar.activation(rstd[:], rvar[:], AF.Sqrt)
    mb = sm.tile([P, TCH], F32, tag="lnb")
    rb = sm.tile([P, TCH], F32, tag="lnb")
    nc.gpsimd.partition_broadcast(mb[:], mean[0:1, :])
    nc.gpsimd.partition_broadcast(rb[:], rstd[0:1, :])
    for k in range(KS):
        if fold_affine:
            # out_bf = pure-normalized n (its g/be are folded host-side into
            # the consuming layer's weights); out_sb = g*n + be (residual)
            t = sm.tile([P, TCH], F32, tag="lnt", bufs=2)
            nc.vector.tensor_tensor(t[:], src_sb[:, k, :], mb[:], ALU.subtract)
            nc.vector.tensor_tensor(out_bf[:, k, :], t[:], rb[:], ALU.mult)
            last = nc.vector.tensor_scalar(
                out=out_sb[:, k, :], in0=out_bf[:, k, :],
                scalar1=g[:, k : k + 1], scalar2=be[:, k : k + 1],
                op0=ALU.mult, op1=ALU.add)
        else:
            nc.vector.tensor_tensor(out_sb[:, k, :], src_sb[:, k, :], mb[:],
                                    ALU.subtract)
            nc.vector.tensor_tensor(out_sb[:, k, :], out_sb[:, k, :], rb[:],
                                    ALU.mult)
            last = nc.vector.tensor_scalar(
                out=out_sb[:, k, :], in0=out_sb[:, k, :],
                scalar1=g[:, k : k + 1], scalar2=be[:, k : k + 1],
                op0=ALU.mult, op1=ALU.add)
            if out_bf is not None:
                last = nc.vector.tensor_copy(out=out_bf[:, k, :],
                                             in_=out_sb[:, k, :])
    return last


# ======================= host side =======================

_CACHE = {}


def _fm(a):
    """[T, D] -> feature-major [128, KS, T]."""
    T = a.shape[0]
    return np.ascontiguousarray(a.T.reshape(KS, P, T).transpose(1, 0, 2))


def _wfm(w):
    """[D_in, D_out] -> [128, D_in//128, D_out]."""
    return np.ascontiguousarray(w.reshape(-1, P, w.shape[1]).transpose(1, 0, 2))


def _pbias(b):
    return np.ascontiguousarray(b.reshape(-1, P).T.astype(np.float32))


def _templates():
    """Per-pair additive score templates [sr, pair, 4*128]:
    [A(even) | B(even) | A(odd) | B(odd)] along the last axis."""
    slopes = (2.0 ** (-8.0 * np.arange(1, H + 1) / H)).astype(np.float64)
    sr = np.arange(P)[:, None]
    tr = np.arange(P)[None, :]
    dA = sr - tr
    dB = sr + P - tr
    A = np.where((dA >= 1) & (dA <= 64),
                 (dA - 64)[None] * slopes[:, None, None], NEG)
    Bt = np.where((dB >= 1) & (dB <= 64),
                  (dB - 64)[None] * slopes[:, None, None], NEG)
    A0 = A.copy()
    A0[:, :64, :] = NEG

    def pack(Ax):
        blocks = [np.concatenate([Ax[2 * j], Bt[2 * j], Ax[2 * j + 1],
                                  Bt[2 * j + 1]], axis=1)
                  for j in range(H // 2)]  # each [sr, 512]
        t = np.stack(blocks, axis=0)  # [8, sr, 512]
        return np.ascontiguousarray(t.transpose(1, 0, 2).astype(np.float32))

    return pack(A), pack(A0)


def kernel(**inputs):
    bf = ml_dtypes.bfloat16
    x = np.asarray(inputs["x"], np.float32)
    mem = np.asarray(inputs["mem"], np.float32)
    g = lambda n: np.asarray(inputs[n], np.float32)

    tP, tP0 = _templates()
    CH = L // TCH

    bias_list = {
        "sbq": g("sbq") / 8.0, "sbk": g("sbk"),
        "sbo": g("sbo") + g("sbv") @ g("swo"),
        "cbq": (g("cbq") + g("be1") @ g("cwq")) / 8.0,
        "cbk": g("cbk"), "cbo": g("cbo") + g("cbv") @ g("cwo"),
        "b2": g("b2"),
        "g1": g("g1"), "be1": g("be1"), "g2": g("g2"), "be2": g("be2"),
        "g3": g("g3"), "be3": g("be3"),
    }
    biases = np.concatenate([_pbias(bias_list[n]) for n in NBIAS]
                            + [_pbias(g("b1"))], axis=1)

    shared = {
        "swq": _wfm(g("swq") / 8.0).astype(bf), "swk": _wfm(g("swk")).astype(bf),
        "swv": _wfm(g("swv")).astype(bf), "swo": _wfm(g("swo")).astype(bf),
        "cwq": _wfm(g("g1")[:, None] * g("cwq") / 8.0).astype(bf),
        "cwo": _wfm(g("cwo")).astype(bf),
        "w1": _wfm(g("g2")[:, None] * g("w1")).astype(bf),
        "w2": _wfm(g("w2")).astype(bf),
        "biases": np.ascontiguousarray(biases),
        "tmplP": tP.astype(bf),
    }
    cwk_fm = _wfm(g("cwk")).astype(bf)
    cwv_fm = _wfm(g("cwv")).astype(bf)
    cbk_p = _pbias(g("cbk"))
    mem_fm = [np.ascontiguousarray(_fm(mem[b]).astype(bf)) for b in range(B)]

    in_maps = []
    for core in range(8):
        b, c = core // CH, core % CH
        t0 = c * TCH
        xpad = np.zeros((HALO + TCH, D), np.float32)
        lo = max(0, t0 - HALO)
        xpad[HALO - (t0 - lo):] = x[b, lo : t0 + TCH]
        m = dict(shared)
        m["memf"] = mem_fm[b]
        m["xb"] = np.ascontiguousarray(_fm(xpad).astype(bf))
        m["xf"] = np.ascontiguousarray(_fm(x[b, t0 : t0 + TCH]))
        m["tmplP0"] = (tP0 if c == 0 else tP).astype(bf)
        # this core's 4 cross-attn heads (feature cols [256c, 256c+256))
        m["cwk_my"] = np.ascontiguousarray(cwk_fm[:, :, c * 256 : (c + 1) * 256])
        m["cwv_my"] = np.ascontiguousarray(cwv_fm[:, :, c * 256 : (c + 1) * 256])
        m["cbk_my"] = np.ascontiguousarray(cbk_p[:, 2 * c : 2 * c + 2])
        in_maps.append(m)

    if "nc" not in _CACHE:
        nc = bacc.Bacc("TRN2", target_bir_lowering=False, debug=False,
                       num_devices=8)
        build(nc)
        nc.compile()
        _CACHE["nc"] = nc
    nc = _CACHE["nc"]

    res = run_bass_kernel_spmd(nc, in_maps, core_ids=list(range(8)),
                               **_CACHE.get("run_kwargs", {}))
    _CACHE["last"] = res

    y = np.empty((B, L, D), np.float32)
    for core in range(8):
        b, c = core // CH, core % CH
        o = np.asarray(res.results[core]["out"])  # [128, KS, TCH]
        y[b, c * TCH : (c + 1) * TCH, :] = o.transpose(1, 0, 2).reshape(D, TCH).T
    return y
